# revision 8
# baseline (speedup 1.0000x reference)
"""Trainium2 Bass kernel for the RNN-T JointNetwork problem.

  enc = h_enc @ W_enc + b_enc            (B,T,1,J)
  dec = h_dec @ W_dec                    (B,1,U,J)
  z   = tanh(enc + dec)                  (B,T,U,J)
  out = z @ W_out + b_out                (B,T,U,V)

Shapes: B=4, T=256, U=64, D=J=V=512, fp32 in/out.

Sharding: 8 cores, data parallel over (B x T/2): core c handles batch
b = c//2 and t-half th = c%2 (TH=128 t values). Params replicated.

Per-core dataflow (all operands bf16; J on the partition dim):
  inputs arrive pre-packed as [128, k*W+x] (k = contraction chunk) so each
  tensor is ONE big DMA; loads are spread over the sync/scalar/gpsimd
  issue queues so issue latency doesn't serialize.
  encT[j,t] = W_enc^T @ h_encT  (+ b_enc per-partition), bf16 [128, 4*TH]
  dec_rep[j, u*16+k] = (W_dec^T @ h_decT)[j,u] replicated x16 along free
      so the broadcast-add below runs in DVE 2x mode (both tensor_tensor
      operands get innermost stride 1, 16-bit).
  loop over NG=8 groups of TG=16 t's; z free index is (u, ti):
    zp[j, u*16+ti] = dec_rep + encT[:, g*16+ti] bcast  (DVE TT bf16 2x)
    zt = tanh(zp)                                      (one ACT instr)
    for each vb (4 chunks of V):
      psum[v128, 1024] += W_out[jc,vb]^T @ zt[jc]      (PE, bf16, W stationary)
      out_sbuf = psum + b_out[vb] per-partition        (DVE vb<2, ACT vb>=2)
      DMA out_sbuf -> outT[vb*128:, g*1024:]           (fp32)

Output leaves the device transposed as outT [V, TH*U] with columns
ordered (g, u, ti); the host reshapes/transposes back to (TH, U, V).
"""

import numpy as np

B, T, U = 4, 256, 64
D, J, V = 512, 512, 512
NCORES = 8
TH = T // 2          # t's per core = 128
KC = 4               # 512/128 contraction chunks
TG = 16              # t's per group
NG = TH // TG        # 8 groups
GW = TG * U          # z columns per group = 1024

_compiled = None


def _build():
    import concourse.bass as bass
    import concourse.tile as tile
    from concourse import mybir

    fp32 = mybir.dt.float32
    bf16 = mybir.dt.bfloat16
    AF = mybir.ActivationFunctionType

    nc = bass.Bass()

    henct = nc.declare_dram_parameter("henct", [128, KC * TH], bf16, isOutput=False)
    hdect = nc.declare_dram_parameter("hdect", [128, KC * U], bf16, isOutput=False)
    wenc = nc.declare_dram_parameter("wenc", [128, KC * J], bf16, isOutput=False)
    wdec = nc.declare_dram_parameter("wdec", [128, KC * J], bf16, isOutput=False)
    wout = nc.declare_dram_parameter("wout", [128, KC * V], bf16, isOutput=False)
    benc = nc.declare_dram_parameter("benc", [128, KC], fp32, isOutput=False)
    bout = nc.declare_dram_parameter("bout", [128, KC], fp32, isOutput=False)
    out = nc.declare_dram_parameter("out", [V, TH * U], fp32, isOutput=True)

    with tile.TileContext(nc) as tc:
        with (
            tc.tile_pool(name="const", bufs=1) as const,
            tc.tile_pool(name="zp", bufs=3) as zp_pool,
            tc.tile_pool(name="zt", bufs=3) as zt_pool,
            tc.tile_pool(name="outs", bufs=4) as outs_pool,
            tc.tile_pool(name="ps", bufs=4, space="PSUM") as ps,
        ):
            # ---- load inputs over 3 issue queues; weights arrive in
            # k-chunks so the setup matmuls start on partial data ----
            henct_s = const.tile([128, KC * TH], bf16, tag="henct")
            nc.sync.dma_start(henct_s[:], henct[:])
            wenc_s = const.tile([128, KC * J], bf16, tag="wenc")
            wenc_c = []
            for k in range(KC):
                wenc_c.append(wenc_s[:, k * J:(k + 1) * J])
                nc.sync.dma_start(wenc_c[k], wenc[:, k * J:(k + 1) * J])
            hdect_s = const.tile([128, KC * U], bf16, tag="hdect")
            nc.scalar.dma_start(hdect_s[:], hdect[:])
            wdec_s = const.tile([128, KC * J], bf16, tag="wdec")
            wdec_c = []
            for k in range(KC):
                wdec_c.append(wdec_s[:, k * J:(k + 1) * J])
                nc.scalar.dma_start(wdec_c[k], wdec[:, k * J:(k + 1) * J])
            wout_s = const.tile([128, KC * V], bf16, tag="wout")
            nc.gpsimd.dma_start(wout_s[:], wout[:])
            benc_s = const.tile([128, KC], fp32, tag="benc")
            nc.gpsimd.dma_start(benc_s[:], benc[:])
            bout_s = const.tile([128, KC], fp32, tag="bout")
            nc.gpsimd.dma_start(bout_s[:], bout[:])

            # ---- encT / dec_rep (setup matmuls, bf16, k outermost so each
            # weight chunk is consumed as soon as its DMA lands) ----
            encT_all = const.tile([128, KC * TH], bf16, tag="encT")
            dec_rep = const.tile([128, KC * GW], bf16, tag="dec_rep")
            pes = [ps.tile([128, GW], fp32, tag="ps", name=f"pe{j}") for j in range(KC)]
            for k in range(KC):
                for jc in range(KC):
                    nc.tensor.matmul(
                        pes[jc][:, :TH],
                        wenc_c[k][:, jc * 128:(jc + 1) * 128],
                        henct_s[:, k * TH:(k + 1) * TH],
                        start=(k == 0),
                        stop=(k == KC - 1),
                    )
            for jc in range(KC):
                nc.vector.tensor_scalar_add(
                    encT_all[:, jc * TH:(jc + 1) * TH], pes[jc][:, :TH],
                    benc_s[:, jc:jc + 1],
                )
            pds = [ps.tile([128, GW], fp32, tag="ps", name=f"pd{j}") for j in range(KC)]
            for k in range(KC):
                for jc in range(KC):
                    nc.tensor.matmul(
                        pds[jc][:, :U],
                        wdec_c[k][:, jc * 128:(jc + 1) * 128],
                        hdect_s[:, k * U:(k + 1) * U],
                        start=(k == 0),
                        stop=(k == KC - 1),
                    )
            for jc in range(KC):
                # dec_rep[j, u*16+k] = dec[j, u] directly from PSUM
                nc.vector.tensor_copy(
                    dec_rep[:, jc * GW:(jc + 1) * GW].rearrange(
                        "p (u k) -> p u k", u=U
                    ),
                    pds[jc][:, :U]
                    .rearrange("p (u x) -> p u x", x=1)
                    .to_broadcast([128, U, TG]),
                )

            # ---- main loop over t-groups ----
            def make_z(g):
                zp = zp_pool.tile([128, KC * GW], bf16, tag="zp")
                for jc in range(KC):
                    # zp[j, u*16+ti] = dec_rep[j, u*16+ti] + encT[j, g*16+ti]
                    nc.vector.tensor_add(
                        zp[:, jc * GW:(jc + 1) * GW].rearrange(
                            "p (u k) -> p u k", u=U
                        ),
                        dec_rep[:, jc * GW:(jc + 1) * GW].rearrange(
                            "p (u k) -> p u k", u=U
                        ),
                        encT_all[:, jc * TH + g * TG: jc * TH + (g + 1) * TG]
                        .rearrange("p (x k) -> p x k", x=1)
                        .to_broadcast([128, U, TG]),
                    )
                zt = zt_pool.tile([128, KC * GW], bf16, tag="zt")
                if g == 0:
                    # finer tanh granularity lets the first matmuls start
                    # as soon as the jc=0 chunk is ready
                    for jc in range(KC):
                        nc.scalar.activation(
                            zt[:, jc * GW:(jc + 1) * GW],
                            zp[:, jc * GW:(jc + 1) * GW], AF.Tanh,
                        )
                else:
                    nc.scalar.activation(zt[:], zp[:], AF.Tanh)
                return zt

            zt = make_z(0)
            for g in range(NG):
                zt_next = make_z(g + 1) if g + 1 < NG else None

                for vb in range(KC):
                    po = ps.tile([128, GW], fp32, tag="ps")
                    for jc in range(KC):
                        lhsT = wout_s[:, jc * V + vb * 128: jc * V + (vb + 1) * 128]
                        for h in range(2):
                            nc.tensor.matmul(
                                po[:, h * 512:(h + 1) * 512],
                                lhsT,
                                zt[:, jc * GW + h * 512: jc * GW + (h + 1) * 512],
                                start=(jc == 0),
                                stop=(jc == KC - 1),
                            )
                    ob = outs_pool.tile([128, GW], fp32, tag="ob")
                    if g == NG - 1:
                        # tail: split each drain across both engines so the
                        # kernel's critical path ends sooner
                        nc.vector.tensor_scalar_add(
                            ob[:, :512], po[:, :512], bout_s[:, vb:vb + 1]
                        )
                        nc.scalar.add(
                            ob[:, 512:], po[:, 512:], bout_s[:, vb:vb + 1]
                        )
                        nc.sync.dma_start(
                            out[vb * 128:(vb + 1) * 128,
                                g * GW:g * GW + 512],
                            ob[:, :512],
                        )
                        nc.scalar.dma_start(
                            out[vb * 128:(vb + 1) * 128,
                                g * GW + 512:(g + 1) * GW],
                            ob[:, 512:],
                        )
                    else:
                        if vb < 2:
                            nc.vector.tensor_scalar_add(
                                ob[:], po[:], bout_s[:, vb:vb + 1]
                            )
                        else:
                            nc.scalar.add(ob[:], po[:], bout_s[:, vb:vb + 1])
                        nc.sync.dma_start(
                            out[vb * 128:(vb + 1) * 128, g * GW:(g + 1) * GW],
                            ob[:],
                        )
                zt = zt_next

    _split_multi_waits(nc)
    return nc


_COMPUTE_OPS = {
    "Matmult", "Ldweights", "TensorTensor", "TensorCopy", "TensorScalarPtr",
    "Activation", "TensorReduce", "Memset", "ScalarTensorTensor",
    "TensorScalar", "DMACopy", "Drain", "EventSemaphore",
}


def _split_multi_waits(nc):
    """walrus codegen in this container allows a single sync-wait command
    per TPB compute instruction; Tile emits several.  Hoist all but one
    wait onto standalone EventSemaphore instructions placed just before
    the offending instruction (same engine, so semantics are identical).
    """
    from concourse import mybir

    ctr = [0]
    for fn in nc.m.functions:
        for blk in fn.blocks:
            insts = blk.instructions
            out = []
            for inst in insts:
                si = getattr(inst, "sync_info", None)
                ow = list(si.on_wait) if si and si.on_wait else []
                if (
                    len(ow) > 1
                    and getattr(inst, "opcode", None) in _COMPUTE_OPS
                ):
                    for w in ow[:-1]:
                        ctr[0] += 1
                        ev = mybir.InstEventSemaphore(
                            name=f"WS-{ctr[0]}-{inst.name}",
                            ins=[],
                            outs=[],
                            sync_info=mybir.SyncInfo(
                                on_wait=[w], on_update=[]
                            ),
                        )
                        ev.engine = inst.engine
                        out.append(ev)
                    inst.sync_info = mybir.SyncInfo(
                        on_wait=[ow[-1]], on_update=list(si.on_update or [])
                    )
                out.append(inst)
            blk.instructions = out


def _get_compiled():
    global _compiled
    if _compiled is None:
        _compiled = _build()
    return _compiled


def _pack(a, kc):
    """[kc*128, W] -> [128, kc*W] with block k at columns [k*W:(k+1)*W]."""
    w = a.shape[1]
    return np.ascontiguousarray(
        a.reshape(kc, 128, w).transpose(1, 0, 2).reshape(128, kc * w)
    )


def kernel(h_enc, h_dec, W_enc, b_enc, W_dec, W_out, b_out, **_):
    nc = _get_compiled()
    import ml_dtypes
    from concourse.bass_utils import run_bass_kernel_spmd

    bfl = ml_dtypes.bfloat16
    h_enc = np.asarray(h_enc, dtype=np.float32)
    h_dec = np.asarray(h_dec, dtype=np.float32)
    wenc_p = _pack(np.asarray(W_enc, dtype=np.float32).astype(bfl), KC)
    wdec_p = _pack(np.asarray(W_dec, dtype=np.float32).astype(bfl), KC)
    wout_p = _pack(np.asarray(W_out, dtype=np.float32).astype(bfl), KC)
    benc_cols = np.ascontiguousarray(
        np.asarray(b_enc, dtype=np.float32).reshape(KC, 128).T
    )
    bout_cols = np.ascontiguousarray(
        np.asarray(b_out, dtype=np.float32).reshape(KC, 128).T
    )

    in_maps = []
    for c in range(NCORES):
        b, th = c // 2, c % 2
        henct = _pack(
            h_enc[b, th * TH:(th + 1) * TH, 0, :].T.astype(bfl), KC
        )  # (128, 4*TH)
        hdect = _pack(h_dec[b, 0, :, :].T.astype(bfl), KC)  # (128, 4*U)
        in_maps.append(
            {
                "henct": henct,
                "hdect": hdect,
                "wenc": wenc_p,
                "wdec": wdec_p,
                "wout": wout_p,
                "benc": benc_cols,
                "bout": bout_cols,
            }
        )

    global _last_in_maps
    _last_in_maps = in_maps
    res = run_bass_kernel_spmd(nc, in_maps, list(range(NCORES)))

    out_full = np.empty((B, T, U, V), dtype=np.float32)
    for c in range(NCORES):
        b, th = c // 2, c % 2
        o = res.results[c]["out"]  # (V, TH*U), cols ordered (g, u, ti)
        o = o.reshape(V, NG, U, TG).transpose(1, 3, 2, 0)  # (NG, TG, U, V)
        out_full[b, th * TH:(th + 1) * TH] = o.reshape(TH, U, V)
    return out_full


# revision 10
# speedup vs baseline: 1.0296x; 1.0296x over previous
"""Trainium2 Bass kernel for the RNN-T JointNetwork problem.

  enc = h_enc @ W_enc + b_enc            (B,T,1,J)
  dec = h_dec @ W_dec                    (B,1,U,J)
  z   = tanh(enc + dec)                  (B,T,U,J)
  out = z @ W_out + b_out                (B,T,U,V)

Shapes: B=4, T=256, U=64, D=J=V=512, fp32 in/out.

Sharding: 8 cores, data parallel over (B x T/2): core c handles batch
b = c//2 and t-half th = c%2 (TH=128 t values). Params replicated.

Per-core dataflow (all operands bf16; J on the partition dim):
  inputs arrive pre-packed as [128, k*W+x] (k = contraction chunk) so each
  tensor is ONE big DMA; loads are spread over the sync/scalar/gpsimd
  issue queues so issue latency doesn't serialize.
  encT[j,t] = W_enc^T @ h_encT  (+ b_enc per-partition), bf16 [128, 4*TH]
  dec_rep[j, u*16+k] = (W_dec^T @ h_decT)[j,u] replicated x16 along free
      so the broadcast-add below runs in DVE 2x mode (both tensor_tensor
      operands get innermost stride 1, 16-bit).
  loop over NG=8 groups of TG=16 t's; z free index is (u, ti):
    zp[j, u*16+ti] = dec_rep + encT[:, g*16+ti] bcast  (DVE TT bf16 2x)
    zt = tanh(zp)                                      (one ACT instr)
    for each vb (4 chunks of V):
      psum[v128, 1024] += W_out[jc,vb]^T @ zt[jc]      (PE, bf16, W stationary)
      out_sbuf = psum + b_out[vb] per-partition        (DVE vb<2, ACT vb>=2)
      DMA out_sbuf -> outT[vb*128:, g*1024:]           (fp32)

Output leaves the device transposed as outT [V, TH*U] with columns
ordered (g, u, ti); the host reshapes/transposes back to (TH, U, V).
"""

import numpy as np

B, T, U = 4, 256, 64
D, J, V = 512, 512, 512
NCORES = 8
TH = T // 2          # t's per core = 128
KC = 4               # 512/128 contraction chunks
TG = 16              # t's per group
NG = TH // TG        # 8 groups
GW = TG * U          # z columns per group = 1024

_compiled = None


def _build():
    import concourse.bass as bass
    import concourse.tile as tile
    from concourse import mybir

    fp32 = mybir.dt.float32
    bf16 = mybir.dt.bfloat16
    AF = mybir.ActivationFunctionType

    nc = bass.Bass()

    henct = nc.declare_dram_parameter("henct", [128, KC * TH], bf16, isOutput=False)
    hdect = nc.declare_dram_parameter("hdect", [128, KC * U], bf16, isOutput=False)
    wenc = nc.declare_dram_parameter("wenc", [128, KC * J], bf16, isOutput=False)
    wdec = nc.declare_dram_parameter("wdec", [128, KC * J], bf16, isOutput=False)
    wout = nc.declare_dram_parameter("wout", [128, KC * V], bf16, isOutput=False)
    benc = nc.declare_dram_parameter("benc", [128, KC], fp32, isOutput=False)
    bout = nc.declare_dram_parameter("bout", [128, KC], fp32, isOutput=False)
    out = nc.declare_dram_parameter("out", [V, TH * U], fp32, isOutput=True)

    with tile.TileContext(nc) as tc:
        with (
            tc.tile_pool(name="const", bufs=1) as const,
            tc.tile_pool(name="zp", bufs=3) as zp_pool,
            tc.tile_pool(name="zt", bufs=3) as zt_pool,
            tc.tile_pool(name="outs", bufs=4) as outs_pool,
            tc.tile_pool(name="ps", bufs=4, space="PSUM") as ps,
        ):
            # ---- load inputs: one DMA per tensor, 3 issue queues ----
            henct_s = const.tile([128, KC * TH], bf16, tag="henct")
            nc.sync.dma_start(henct_s[:], henct[:])
            wenc_s = const.tile([128, KC * J], bf16, tag="wenc")
            nc.sync.dma_start(wenc_s[:], wenc[:])
            hdect_s = const.tile([128, KC * U], bf16, tag="hdect")
            nc.scalar.dma_start(hdect_s[:], hdect[:])
            wdec_s = const.tile([128, KC * J], bf16, tag="wdec")
            nc.scalar.dma_start(wdec_s[:], wdec[:])
            wout_s = const.tile([128, KC * V], bf16, tag="wout")
            nc.gpsimd.dma_start(wout_s[:], wout[:])
            benc_s = const.tile([128, KC], fp32, tag="benc")
            nc.gpsimd.dma_start(benc_s[:], benc[:])
            bout_s = const.tile([128, KC], fp32, tag="bout")
            nc.gpsimd.dma_start(bout_s[:], bout[:])

            # ---- encT / dec_rep (setup matmuls, bf16); group-0 z is
            # produced per-jc inline so the main matmuls start early ----
            encT_all = const.tile([128, KC * TH], bf16, tag="encT")
            dec_rep = const.tile([128, KC * GW], bf16, tag="dec_rep")
            zp0 = zp_pool.tile([128, KC * GW], bf16, tag="zp")
            zt0 = zt_pool.tile([128, KC * GW], bf16, tag="zt")

            def zslice(x, jc):
                return x[:, jc * GW:(jc + 1) * GW].rearrange(
                    "p (u k) -> p u k", u=U
                )

            def enc_bcast(g, jc):
                return (
                    encT_all[:, jc * TH + g * TG: jc * TH + (g + 1) * TG]
                    .rearrange("p (x k) -> p x k", x=1)
                    .to_broadcast([128, U, TG])
                )

            for jc in range(KC):
                pe = ps.tile([128, GW], fp32, tag="ps")
                for k in range(KC):
                    nc.tensor.matmul(
                        pe[:, :TH],
                        wenc_s[:, k * J + jc * 128: k * J + (jc + 1) * 128],
                        henct_s[:, k * TH:(k + 1) * TH],
                        start=(k == 0),
                        stop=(k == KC - 1),
                    )
                nc.vector.tensor_scalar_add(
                    encT_all[:, jc * TH:(jc + 1) * TH], pe[:, :TH],
                    benc_s[:, jc:jc + 1],
                )

                pd = ps.tile([128, GW], fp32, tag="ps")
                for k in range(KC):
                    nc.tensor.matmul(
                        pd[:, :U],
                        wdec_s[:, k * J + jc * 128: k * J + (jc + 1) * 128],
                        hdect_s[:, k * U:(k + 1) * U],
                        start=(k == 0),
                        stop=(k == KC - 1),
                    )
                # dec_rep[j, u*16+k] = dec[j, u] directly from PSUM
                nc.vector.tensor_copy(
                    zslice(dec_rep, jc),
                    pd[:, :U]
                    .rearrange("p (u x) -> p u x", x=1)
                    .to_broadcast([128, U, TG]),
                )
                # group-0 z for this jc chunk, immediately
                nc.vector.tensor_add(
                    zslice(zp0, jc), zslice(dec_rep, jc), enc_bcast(0, jc)
                )
                nc.scalar.activation(
                    zt0[:, jc * GW:(jc + 1) * GW],
                    zp0[:, jc * GW:(jc + 1) * GW], AF.Tanh,
                )

            # ---- main loop over t-groups ----
            def make_z(g):
                zp = zp_pool.tile([128, KC * GW], bf16, tag="zp")
                for jc in range(KC):
                    # zp[j, u*16+ti] = dec_rep[j, u*16+ti] + encT[j, g*16+ti]
                    nc.vector.tensor_add(
                        zslice(zp, jc), zslice(dec_rep, jc), enc_bcast(g, jc)
                    )
                zt = zt_pool.tile([128, KC * GW], bf16, tag="zt")
                nc.scalar.activation(zt[:], zp[:], AF.Tanh)
                return zt

            zt = zt0
            for g in range(NG):
                zt_next = make_z(g + 1) if g + 1 < NG else None

                for vb in range(KC):
                    po = ps.tile([128, GW], fp32, tag="ps")
                    for jc in range(KC):
                        lhsT = wout_s[:, jc * V + vb * 128: jc * V + (vb + 1) * 128]
                        for h in range(2):
                            nc.tensor.matmul(
                                po[:, h * 512:(h + 1) * 512],
                                lhsT,
                                zt[:, jc * GW + h * 512: jc * GW + (h + 1) * 512],
                                start=(jc == 0),
                                stop=(jc == KC - 1),
                            )
                    ob = outs_pool.tile([128, GW], fp32, tag="ob")
                    if g == NG - 1:
                        # tail: split each drain across both engines so the
                        # kernel's critical path ends sooner
                        nc.vector.tensor_scalar_add(
                            ob[:, :512], po[:, :512], bout_s[:, vb:vb + 1]
                        )
                        nc.scalar.add(
                            ob[:, 512:], po[:, 512:], bout_s[:, vb:vb + 1]
                        )
                        nc.sync.dma_start(
                            out[vb * 128:(vb + 1) * 128,
                                g * GW:g * GW + 512],
                            ob[:, :512],
                        )
                        nc.scalar.dma_start(
                            out[vb * 128:(vb + 1) * 128,
                                g * GW + 512:(g + 1) * GW],
                            ob[:, 512:],
                        )
                    else:
                        # drains: DVE gets 2 or 3 of the 4 per group
                        # (alternating) so neither DVE nor ACT saturates
                        ndve = 2 + (g % 2)
                        if vb < ndve:
                            nc.vector.tensor_scalar_add(
                                ob[:], po[:], bout_s[:, vb:vb + 1]
                            )
                        else:
                            nc.scalar.add(ob[:], po[:], bout_s[:, vb:vb + 1])
                        nc.sync.dma_start(
                            out[vb * 128:(vb + 1) * 128, g * GW:(g + 1) * GW],
                            ob[:],
                        )
                zt = zt_next

    _split_multi_waits(nc)
    return nc


_COMPUTE_OPS = {
    "Matmult", "Ldweights", "TensorTensor", "TensorCopy", "TensorScalarPtr",
    "Activation", "TensorReduce", "Memset", "ScalarTensorTensor",
    "TensorScalar", "DMACopy", "Drain", "EventSemaphore",
}


def _split_multi_waits(nc):
    """walrus codegen in this container allows a single sync-wait command
    per TPB compute instruction; Tile emits several.  Hoist all but one
    wait onto standalone EventSemaphore instructions placed just before
    the offending instruction (same engine, so semantics are identical).
    """
    from concourse import mybir

    ctr = [0]
    for fn in nc.m.functions:
        for blk in fn.blocks:
            insts = blk.instructions
            out = []
            for inst in insts:
                si = getattr(inst, "sync_info", None)
                ow = list(si.on_wait) if si and si.on_wait else []
                if (
                    len(ow) > 1
                    and getattr(inst, "opcode", None) in _COMPUTE_OPS
                ):
                    for w in ow[:-1]:
                        ctr[0] += 1
                        ev = mybir.InstEventSemaphore(
                            name=f"WS-{ctr[0]}-{inst.name}",
                            ins=[],
                            outs=[],
                            sync_info=mybir.SyncInfo(
                                on_wait=[w], on_update=[]
                            ),
                        )
                        ev.engine = inst.engine
                        out.append(ev)
                    inst.sync_info = mybir.SyncInfo(
                        on_wait=[ow[-1]], on_update=list(si.on_update or [])
                    )
                out.append(inst)
            blk.instructions = out


def _get_compiled():
    global _compiled
    if _compiled is None:
        _compiled = _build()
    return _compiled


def _pack(a, kc):
    """[kc*128, W] -> [128, kc*W] with block k at columns [k*W:(k+1)*W]."""
    w = a.shape[1]
    return np.ascontiguousarray(
        a.reshape(kc, 128, w).transpose(1, 0, 2).reshape(128, kc * w)
    )


def kernel(h_enc, h_dec, W_enc, b_enc, W_dec, W_out, b_out, **_):
    nc = _get_compiled()
    import ml_dtypes
    from concourse.bass_utils import run_bass_kernel_spmd

    bfl = ml_dtypes.bfloat16
    h_enc = np.asarray(h_enc, dtype=np.float32)
    h_dec = np.asarray(h_dec, dtype=np.float32)
    wenc_p = _pack(np.asarray(W_enc, dtype=np.float32).astype(bfl), KC)
    wdec_p = _pack(np.asarray(W_dec, dtype=np.float32).astype(bfl), KC)
    wout_p = _pack(np.asarray(W_out, dtype=np.float32).astype(bfl), KC)
    benc_cols = np.ascontiguousarray(
        np.asarray(b_enc, dtype=np.float32).reshape(KC, 128).T
    )
    bout_cols = np.ascontiguousarray(
        np.asarray(b_out, dtype=np.float32).reshape(KC, 128).T
    )

    in_maps = []
    for c in range(NCORES):
        b, th = c // 2, c % 2
        henct = _pack(
            h_enc[b, th * TH:(th + 1) * TH, 0, :].T.astype(bfl), KC
        )  # (128, 4*TH)
        hdect = _pack(h_dec[b, 0, :, :].T.astype(bfl), KC)  # (128, 4*U)
        in_maps.append(
            {
                "henct": henct,
                "hdect": hdect,
                "wenc": wenc_p,
                "wdec": wdec_p,
                "wout": wout_p,
                "benc": benc_cols,
                "bout": bout_cols,
            }
        )

    global _last_in_maps
    _last_in_maps = in_maps
    res = run_bass_kernel_spmd(nc, in_maps, list(range(NCORES)))

    out_full = np.empty((B, T, U, V), dtype=np.float32)
    for c in range(NCORES):
        b, th = c // 2, c % 2
        o = res.results[c]["out"]  # (V, TH*U), cols ordered (g, u, ti)
        o = o.reshape(V, NG, U, TG).transpose(1, 3, 2, 0)  # (NG, TG, U, V)
        out_full[b, th * TH:(th + 1) * TH] = o.reshape(TH, U, V)
    return out_full
